# revision 1
# baseline (speedup 1.0000x reference)
"""BERT+CRF loss (torchcrf-style, reduction=sum) on 8 Trainium2 NeuronCores.

Strategy (pure data parallel, batch sharded 8 ways, 8 sequences per core):
  emissions^T = W^T @ X^T on TensorE (X pre-transposed on host, f32)
  CRF forward recurrence in exp space:
      v_t = (v_{t-1}^T expT) * E_t,  E_t = exp(em_t)
  Adjacent steps are paired into 9x9 transfer matrices
      B_p[i,j] = sum_k expT[i,k] E_{2p+1}[k] expT[k,j] E_{2p+2}[j]
  computed on TensorE as  outer(E_a, E_b) [81] x G4 [81,81]  (G4 is a host
  constant built from exp(trans)).  Each sequence's 255 pair matrices are
  split into 16 chunks of 16; a chunk-parallel matrix product runs on
  VectorE with 128 partitions = 8 batches x 16 chunks, 16 steps, periodic
  max-normalization for range safety.  Host combines the 16 chunk matrices
  per sequence (O(B*16*81) f64) and adds the label-indexed numerator terms.
"""

import sys

if "/opt/trn_rl_repo" not in sys.path:
    sys.path.insert(0, "/opt/trn_rl_repo")

import numpy as np

B, S, H, L = 64, 512, 768, 9
NCORES = 8
BPC = B // NCORES          # sequences per core
LL = L * L                 # 81
NPAIR = 256                # pair slots per sequence (255 real + 1 identity)
NCHUNK = 16                # chunks per sequence
SPC = NPAIR // NCHUNK      # pair-steps per chunk = 16
HC = H // 128              # 6 contraction chunks of 128
NORM_STEPS = (5, 11, 15)   # recurrence steps after which we renormalize
NNORM = len(NORM_STEPS)

_CACHE = {}


def _build_bass():
    import concourse.bass as bass
    import concourse.bacc as bacc
    import concourse.mybir as mybir
    import concourse.tile as tile
    from contextlib import ExitStack

    f32 = mybir.dt.float32
    bf16 = mybir.dt.bfloat16
    Alu = mybir.AluOpType
    Act = mybir.ActivationFunctionType
    Ax = mybir.AxisListType

    nc = bacc.Bacc()

    # ---- I/O ----
    xT_d = nc.dram_tensor("xT", [BPC, H, S], f32, kind="ExternalInput")
    w_d = nc.dram_tensor("Wt", [H, L], f32, kind="ExternalInput")
    lab_d = nc.dram_tensor("lab9", [BPC, L, S], f32, kind="ExternalInput")
    g4_d = nc.dram_tensor("G4", [LL, LL], f32, kind="ExternalInput")
    ra_d = nc.dram_tensor("Ra", [L, LL], f32, kind="ExternalInput")
    rb_d = nc.dram_tensor("Rb", [L, LL], f32, kind="ExternalInput")
    iota_d = nc.dram_tensor("iota9", [L, 1], f32, kind="ExternalInput")
    id_d = nc.dram_tensor("id128", [128, LL], f32, kind="ExternalInput")

    s_out = nc.dram_tensor("S_out", [128, LL], f32, kind="ExternalOutput")
    m_out = nc.dram_tensor("m_out", [128, NNORM], f32, kind="ExternalOutput")
    e_out = nc.dram_tensor("e_out", [BPC, L, 2], f32, kind="ExternalOutput")
    nt_out = nc.dram_tensor("nt_out", [L, BPC], f32, kind="ExternalOutput")

    with ExitStack() as ctx:
        tc = ctx.enter_context(tile.TileContext(nc))
        const = ctx.enter_context(tc.tile_pool(name="const", bufs=1))
        xpool = ctx.enter_context(tc.tile_pool(name="x", bufs=3))
        xbpool = ctx.enter_context(tc.tile_pool(name="xb", bufs=2))
        epool = ctx.enter_context(tc.tile_pool(name="e", bufs=2))
        lpool = ctx.enter_context(tc.tile_pool(name="lab", bufs=3))
        spool = ctx.enter_context(tc.tile_pool(name="sm", bufs=3))
        rpool = ctx.enter_context(tc.tile_pool(name="rec", bufs=1))
        dpool = ctx.enter_context(tc.tile_pool(name="dram", bufs=1, space="DRAM"))
        ps_em = ctx.enter_context(tc.tile_pool(name="psem", bufs=3, space="PSUM"))
        ps_rep = ctx.enter_context(tc.tile_pool(name="psrep", bufs=1, space="PSUM"))
        ps_b = ctx.enter_context(tc.tile_pool(name="psb", bufs=2, space="PSUM"))

        # ---- constants into SBUF (matmul operands cast to bf16 by DMA) ----
        w_sb = const.tile([128, HC, L], bf16)
        nc.gpsimd.dma_start(w_sb[:], w_d[:].rearrange("(c k) l -> k c l", c=HC))
        g4_sb = const.tile([LL, LL], bf16)
        nc.gpsimd.dma_start(g4_sb[:], g4_d[:])
        ra_sb = const.tile([L, LL], bf16)
        nc.gpsimd.dma_start(ra_sb[:], ra_d[:])
        rb_sb = const.tile([L, LL], bf16)
        nc.gpsimd.dma_start(rb_sb[:], rb_d[:])
        iota_sb = const.tile([L, 1], f32)
        nc.sync.dma_start(iota_sb[:], iota_d[:])

        # ---- persistent recurrence state ----
        s_tile = rpool.tile([128, LL], f32)            # chunk-product state
        nc.sync.dma_start(s_tile[:], id_d[:])          # init to I (per row)
        bc_tile = rpool.tile([128, SPC * LL], f32)     # pair matrices, chunk layout
        tmp729 = rpool.tile([128, L * L * L], f32)
        mvals = rpool.tile([128, NNORM], f32)
        emtag = rpool.tile([L, BPC], f32)

        # internal DRAM bounce for pair matrices; row 255 of each b = identity
        b_all = dpool.tile([BPC, NPAIR, LL], f32)
        for b in range(BPC):
            nc.scalar.dma_start(b_all[b, NPAIR - 1, :], id_d[0, :])

        for b in range(BPC):
            # stream X^T for this sequence (f32, HWDGE, two queue-spread DMAs),
            # then cast to bf16 on the otherwise-idle GpSimd engine
            xt = xpool.tile([128, HC, S], f32)
            src = xT_d[b].rearrange("(c k) s -> k c s", c=HC)
            nc.sync.dma_start(xt[:, 0 : HC // 2, :], src[:, 0 : HC // 2, :])
            nc.sync.dma_start(xt[:, HC // 2 : HC, :], src[:, HC // 2 : HC, :])
            xtb = xbpool.tile([128, HC, S], bf16)
            nc.vector.tensor_copy(xtb[:], xt[:])

            # emissions^T [9, S] in PSUM (no bias; handled on host)
            em_ps = ps_em.tile([L, S], f32)
            for c in range(HC):
                nc.tensor.matmul(
                    em_ps[:], w_sb[:, c, :], xtb[:, c, :],
                    start=(c == 0), stop=(c == HC - 1),
                )

            # E = exp(em) in bf16, with one extra zero column at index S
            e_sb = epool.tile([L, S + 1], bf16)
            nc.vector.memset(e_sb[:, S : S + 1], 0.0)
            nc.scalar.activation(e_sb[:, 0:S], em_ps[:], Act.Exp)
            # export exp of em columns 0 and S-1 in f32 for host (v0, tail)
            em01 = bass.AP(
                em_ps.tensor, em_ps[:].offset, [[em_ps[:].ap[0][0], L], [S - 1, 2]]
            )
            e01 = spool.tile([L, 2], f32)
            nc.scalar.activation(e01[:], em01, Act.Exp)
            nc.sync.dma_start(e_out[b], e01[:])

            # numerator: sum_t em[label_t, t] accumulated per (l, b)
            lb = lpool.tile([L, S], f32)
            nc.scalar.dma_start(lb[:], lab_d[b])
            msk = spool.tile([L, S], f32)
            nc.vector.scalar_tensor_tensor(
                out=msk[:], in0=lb[:], scalar=iota_sb[:], in1=em_ps[:],
                op0=Alu.is_equal, op1=Alu.mult,
                accum_out=emtag[:, b : b + 1],
            )

            # replicate E_odd / E_even into [81, 256] via TensorE
            ap0 = e_sb[:].ap[0]
            ea_ap = bass.AP(e_sb.tensor, e_sb[:].offset + 1, [[ap0[0], L], [2, NPAIR]])
            eb_ap = bass.AP(e_sb.tensor, e_sb[:].offset + 2, [[ap0[0], L], [2, NPAIR]])
            earep = ps_rep.tile([LL, NPAIR], f32)
            nc.tensor.matmul(earep[:], ra_sb[:], ea_ap, start=True, stop=True)
            ebrep = ps_rep.tile([LL, NPAIR], f32)
            nc.tensor.matmul(ebrep[:], rb_sb[:], eb_ap, start=True, stop=True)
            eacp = spool.tile([LL, NPAIR], bf16)
            nc.scalar.copy(eacp[:], earep[:])
            ebcp = spool.tile([LL, NPAIR], bf16)
            nc.scalar.copy(ebcp[:], ebrep[:])
            outer = spool.tile([LL, NPAIR], bf16)
            nc.vector.tensor_mul(outer[:], eacp[:], ebcp[:])

            # pair matrices B_p = outer^T @ G4, two halves of 128 pairs
            for h in range(2):
                bp = ps_b.tile([128, LL], f32)
                nc.tensor.matmul(
                    bp[:], outer[:, h * 128 : (h + 1) * 128], g4_sb[:],
                    start=True, stop=True,
                )
                bsb = spool.tile([128, LL], f32)
                nc.scalar.copy(bsb[:], bp[:])
                rows = 128 if h == 0 else 127   # skip pair 255 (stays identity)
                nc.sync.dma_start(
                    b_all[b, h * 128 : h * 128 + rows, :], bsb[0:rows, :]
                )
            # chunk-layout rows for this sequence: partition 16*b+c
            nc.scalar.dma_start(
                bc_tile[16 * b : 16 * (b + 1), :],
                b_all[b].rearrange("(c s) j -> c (s j)", c=NCHUNK),
            )

        # ---- chunk-parallel matrix recurrence: S <- S @ B_s ----
        ncol = 0
        for s in range(SPC):
            bs = bc_tile[:, s * LL : (s + 1) * LL]
            in0 = (
                s_tile[:].rearrange("p (i k) -> p i k", i=L)
                .unsqueeze(2).broadcast_to([128, L, L, L])
            )
            # bc stores B^T (column-major B): inner k is contiguous
            in1 = (
                bs.rearrange("p (j k) -> p j k", j=L)
                .unsqueeze(1).broadcast_to([128, L, L, L])
            )
            t3 = tmp729[:].rearrange("p (i j k) -> p i j k", i=L, j=L)
            nc.vector.tensor_tensor(out=t3, in0=in0, in1=in1, op=Alu.mult)
            nc.vector.tensor_reduce(
                out=s_tile[:], in_=t3, axis=Ax.X, op=Alu.add
            )
            if s in NORM_STEPS:
                mc = mvals[:, ncol : ncol + 1]
                ncol += 1
                nc.vector.reduce_max(mc, s_tile[:], axis=Ax.X)
                rec = spool.tile([128, 1], f32)
                nc.vector.reciprocal(rec[:], mc)
                nc.vector.tensor_scalar_mul(s_tile[:], s_tile[:], rec[:])

        nc.sync.dma_start(s_out[:], s_tile[:])
        nc.sync.dma_start(m_out[:], mvals[:])
        nc.sync.dma_start(nt_out[:], emtag[:])

    if not nc.is_finalized():
        nc.finalize()
    return nc


def _get_nc():
    if "nc" not in _CACHE:
        _CACHE["nc"] = _build_bass()
    return _CACHE["nc"]


def _host_consts(trans):
    expT = np.exp(trans.astype(np.float64)).astype(np.float32)  # [9,9]
    k_idx = np.arange(LL) // L   # row index of the 81-flat (k, jb)
    jb_idx = np.arange(LL) % L
    i_idx = np.arange(LL) // L   # col index of the 81-flat (i, j)
    j_idx = np.arange(LL) % L
    # G4[(k,jb),(i,j)] = expT[i,k] * expT[k,j] * (j == jb)
    g4 = (
        expT[np.ix_(i_idx, k_idx)].T
        * expT[np.ix_(k_idx, j_idx)]
        * (j_idx[None, :] == jb_idx[:, None])
    ).astype(np.float32)
    # store B transposed (column-major) so the recurrence reads contiguously
    g4 = np.ascontiguousarray(g4.reshape(LL, L, L).swapaxes(1, 2).reshape(LL, LL))
    ra = (k_idx[None, :] == np.arange(L)[:, None]).astype(np.float32)   # [9,81]
    rb = (jb_idx[None, :] == np.arange(L)[:, None]).astype(np.float32)  # [9,81]
    iota = np.arange(L, dtype=np.float32).reshape(L, 1)
    id128 = np.tile(np.eye(L, dtype=np.float32).reshape(1, LL), (128, 1))
    return expT, g4, ra, rb, iota, id128


def _numpy_reference(hs, mask, labels, W, bb, st, en, tr):
    # general fallback (only used when attention_mask is not all ones)
    em = hs.astype(np.float64) @ W.astype(np.float64) + bb.astype(np.float64)
    maskb = mask.astype(bool)
    maskf = mask.astype(np.float64)
    em_tag = np.take_along_axis(em, labels[..., None], axis=-1)[..., 0]
    num = st.astype(np.float64)[labels[:, 0]] + em_tag[:, 0]
    trs = tr.astype(np.float64)[labels[:, :-1], labels[:, 1:]]
    num = num + np.sum((trs + em_tag[:, 1:]) * maskf[:, 1:], axis=1)
    last = mask.sum(axis=1).astype(np.int64) - 1
    num = num + en.astype(np.float64)[labels[np.arange(len(labels)), last]]
    alpha = st.astype(np.float64)[None, :] + em[:, 0]
    for t in range(1, em.shape[1]):
        x = alpha[:, :, None] + tr.astype(np.float64)[None, :, :] + em[:, t][:, None, :]
        m = x.max(axis=1, keepdims=True)
        nxt = np.log(np.exp(x - m).sum(axis=1)) + m[:, 0, :]
        alpha = np.where(maskb[:, t][:, None], nxt, alpha)
    x = alpha + en.astype(np.float64)[None, :]
    m = x.max(axis=1, keepdims=True)
    denom = np.log(np.exp(x - m).sum(axis=1)) + m[:, 0]
    return np.asarray((denom - num).sum(), dtype=np.float32)


def kernel(**inputs):
    from concourse import bass_utils

    hs = np.asarray(inputs["hidden_states"], dtype=np.float32)
    mask = np.asarray(inputs["attention_mask"])
    labels = np.asarray(inputs["labels"]).astype(np.int64)
    W = np.asarray(inputs["W"], dtype=np.float32)
    bb = np.asarray(inputs["b"], dtype=np.float32)
    st = np.asarray(inputs["start_trans"], dtype=np.float32)
    en = np.asarray(inputs["end_trans"], dtype=np.float32)
    tr = np.asarray(inputs["trans"], dtype=np.float32)

    if not np.all(mask == 1):
        return _numpy_reference(hs, mask, labels, W, bb, st, en, tr)

    expT, g4, ra, rb, iota, id128 = _host_consts(tr)
    xT = np.ascontiguousarray(hs.transpose(0, 2, 1))            # [B, H, S]
    labf = labels.astype(np.float32)
    lab9 = np.ascontiguousarray(
        np.broadcast_to(labf[:, None, :], (B, L, S))
    )                                                            # [B, 9, S]

    nc = _get_nc()
    in_maps = []
    for k in range(NCORES):
        sl = slice(k * BPC, (k + 1) * BPC)
        in_maps.append(
            {
                "xT": xT[sl],
                "Wt": W,
                "lab9": lab9[sl],
                "G4": g4,
                "Ra": ra,
                "Rb": rb,
                "iota9": iota,
                "id128": id128,
            }
        )
    res = bass_utils.run_bass_kernel_spmd(nc, in_maps, list(range(NCORES)))
    _CACHE["last_results"] = res

    # ---- host combine (f64, tiny) ----
    expT64 = np.exp(tr.astype(np.float64))
    e_end = np.exp(en.astype(np.float64))
    e_sb = np.exp((st + bb).astype(np.float64))
    total = 0.0
    for k in range(NCORES):
        r = res.results[k]
        Sf = r["S_out"].astype(np.float64).reshape(BPC, NCHUNK, L, L)
        mv = r["m_out"].astype(np.float64).reshape(BPC, NCHUNK, NNORM)
        E01 = r["e_out"].astype(np.float64)          # [BPC, 9, 2]
        total -= float(r["nt_out"].astype(np.float64).sum())
        for b in range(BPC):
            v = E01[b, :, 0] * e_sb                  # v0 = exp(em_0 + b + start)
            logacc = 0.0
            for c in range(NCHUNK):
                v = v @ Sf[b, c]
                m = v.max()
                v /= m
                logacc += np.log(m)
            v = (v @ expT64) * E01[b, :, 1]          # tail step t = S-1
            denom = np.log(v @ e_end) + logacc + np.log(mv[b]).sum()
            total += denom
        lb = labels[k * BPC : (k + 1) * BPC]
        total -= float(
            st.astype(np.float64)[lb[:, 0]].sum()
            + en.astype(np.float64)[lb[:, -1]].sum()
            + tr.astype(np.float64)[lb[:, :-1], lb[:, 1:]].sum()
            + bb.astype(np.float64)[lb].sum()
        )
    return np.asarray(total, dtype=np.float32)



# revision 4
# speedup vs baseline: 2.6683x; 2.6683x over previous
"""BERT+CRF loss (torchcrf-style, reduction=sum) on 8 Trainium2 NeuronCores.

Strategy (pure data parallel, batch sharded 8 ways, 8 sequences per core):
  X is quantized to fp8-e4m3 on the host (4x less HBM traffic than f32) and
  streamed once through TensorE with DoubleRow fp8 matmuls (256-deep k-tiles)
  to produce emissions^T [9, 512] per sequence.  The CRF forward recurrence
  is reformulated in exp space: step matrix M_t[i,j] = expT[i,j] * E_t[j]
  with E_t = exp(em_t + b).  Triples of steps collapse into one 9x9 transfer
  matrix via a bilinear host constant G6 [81, 81]:
      T_q[i,j] = sum_{k,l} expT[i,k] Ea[k] expT[k,l] Eb[l] expT[l,j]
  so  M_{3q+1} M_{3q+2} M_{3q+3} = T_q * diag(E_{3q+3}).
  The device ships the 170 T_q matrices per sequence (bf16) plus the diag
  factors; the host multiplies the 9x9 chain in f64 with an order-preserving
  normalized tree reduce (O(B*170*81) work) and adds the label-indexed
  numerator terms (accumulated on-device via a masked-sum vector op).
"""

import os
import sys

if "/opt/trn_rl_repo" not in sys.path:
    sys.path.insert(0, "/opt/trn_rl_repo")

import ml_dtypes
import numpy as np

B, S, H, L = 64, 512, 768, 9
NCORES = 8
BPC = B // NCORES          # sequences per core
LL = L * L                 # 81
NT = 170                   # triples per sequence: steps t=1..510; t=511 on host
HC = H // 128              # 6 h-chunks of 128
NKT = 3                    # DoubleRow k-tiles (256-deep each)
MP = 16                    # DoubleRow needs >=16 weight cols per plane; 9 padded
SCALE_W = 64.0             # W is scaled into fp8 range; exp() unscales

_CACHE = {}


def _build_bass():
    import concourse.bass as bass
    import concourse.bacc as bacc
    import concourse.mybir as mybir
    import concourse.tile as tile
    from contextlib import ExitStack

    f32 = mybir.dt.float32
    bf16 = mybir.dt.bfloat16
    f8 = mybir.dt.float8e4
    Alu = mybir.AluOpType
    Act = mybir.ActivationFunctionType
    DR = mybir.MatmulPerfMode.DoubleRow

    nc = bacc.Bacc()

    # ---- I/O ----
    x8_d = nc.dram_tensor("x8", [BPC, 128, HC, S], f8, kind="ExternalInput")
    w8_d = nc.dram_tensor("w8", [128, HC, MP], f8, kind="ExternalInput")
    lab_d = nc.dram_tensor("lab9", [BPC, L, S], f32, kind="ExternalInput")
    g6_d = nc.dram_tensor("G6", [LL, LL], bf16, kind="ExternalInput")
    ra_d = nc.dram_tensor("Ra", [L, LL], bf16, kind="ExternalInput")
    rb_d = nc.dram_tensor("Rb", [L, LL], bf16, kind="ExternalInput")
    iota_d = nc.dram_tensor("iota9", [L, 1], f32, kind="ExternalInput")
    bias_d = nc.dram_tensor("bias9", [L, 1], f32, kind="ExternalInput")

    t_out = nc.dram_tensor("t_out", [LL, BPC, NT], bf16, kind="ExternalOutput")
    emc_out = nc.dram_tensor("emc_out", [L, BPC, NT], bf16, kind="ExternalOutput")
    sm_out = nc.dram_tensor("sm_out", [L, BPC, 3], f32, kind="ExternalOutput")

    with ExitStack() as ctx:
        tc = ctx.enter_context(tile.TileContext(nc))
        const = ctx.enter_context(tc.tile_pool(name="const", bufs=1))
        xpool = ctx.enter_context(tc.tile_pool(name="x", bufs=3))
        epool = ctx.enter_context(tc.tile_pool(name="e", bufs=2))
        lpool = ctx.enter_context(tc.tile_pool(name="lab", bufs=2))
        spool = ctx.enter_context(tc.tile_pool(name="sm", bufs=2))
        upool = ctx.enter_context(tc.tile_pool(name="u", bufs=2))
        rpool = ctx.enter_context(tc.tile_pool(name="res", bufs=1))
        ps_em = ctx.enter_context(tc.tile_pool(name="psem", bufs=2, space="PSUM"))
        ps_rep = ctx.enter_context(tc.tile_pool(name="psrep", bufs=2, space="PSUM"))
        ps_g6 = ctx.enter_context(tc.tile_pool(name="psg6", bufs=2, space="PSUM"))

        # ---- constants into SBUF ----
        w8_sb = const.tile([128, HC, MP], f8)
        nc.sync.dma_start(w8_sb[:], w8_d[:])
        g6_sb = const.tile([LL, LL], bf16)
        nc.sync.dma_start(g6_sb[:], g6_d[:])
        ra_sb = const.tile([L, LL], bf16)
        nc.scalar.dma_start(ra_sb[:], ra_d[:])
        rb_sb = const.tile([L, LL], bf16)
        nc.scalar.dma_start(rb_sb[:], rb_d[:])
        iota_sb = const.tile([L, 1], f32)
        nc.scalar.dma_start(iota_sb[:], iota_d[:])
        bias_sb = const.tile([L, 1], f32)
        nc.scalar.dma_start(bias_sb[:], bias_d[:])

        # ---- persistent result collect tiles ----
        coll_sb = rpool.tile([LL, BPC, NT], bf16)
        emc_sb = rpool.tile([L, BPC, NT], bf16)
        small_sb = rpool.tile([L, BPC, 3], f32)

        for b in range(BPC):
            # stream this sequence's X^T (fp8, per-partition contiguous 3KB)
            xt = xpool.tile([128, HC, S], f8)
            nc.sync.dma_start(xt[:], x8_d[b])

            # emissions^T [9, S] = (64*W)^T @ X^T via 3 DoubleRow k-tiles
            em_ps = ps_em.tile([MP, S], f32)
            for t in range(NKT):
                nc.tensor.matmul(
                    em_ps[:],
                    w8_sb[:, 2 * t : 2 * t + 2, :],
                    xt[:, 2 * t : 2 * t + 2, :],
                    start=(t == 0), stop=(t == NKT - 1),
                    perf_mode=DR,
                )

            # E = exp(em/64 + bias) in bf16
            e_sb = epool.tile([L, S], bf16)
            nc.scalar.activation(
                e_sb[:], em_ps[0:L, :], Act.Exp, bias=bias_sb[:], scale=1.0 / SCALE_W
            )
            # exp of em columns 0 and S-1 in f32 for host (v0, tail)
            em01 = bass.AP(
                em_ps.tensor, em_ps[:].offset, [[em_ps[:].ap[0][0], L], [S - 1, 2]]
            )
            nc.scalar.activation(
                small_sb[:, b, 0:2], em01, Act.Exp, bias=bias_sb[:], scale=1.0 / SCALE_W
            )
            # diag factors E_{3q+3}, q=0..169 (em columns 3,6,...,510)
            emc_ap = bass.AP(
                em_ps.tensor, em_ps[:].offset + 3, [[em_ps[:].ap[0][0], L], [3, NT]]
            )
            nc.scalar.activation(
                emc_sb[:, b, :], emc_ap, Act.Exp, bias=bias_sb[:], scale=1.0 / SCALE_W
            )

            # numerator: sum_t em_scaled[label_t, t] accumulated per l
            lb = lpool.tile([L, S], f32)
            nc.scalar.dma_start(lb[:], lab_d[b])
            msk = spool.tile([L, S], f32)
            nc.vector.scalar_tensor_tensor(
                out=msk[:], in0=lb[:], scalar=iota_sb[:], in1=em_ps[0:L, :],
                op0=Alu.is_equal, op1=Alu.mult,
                accum_out=small_sb[:, b, 2:3],
            )

            # u[(k,l), q] = Ea[k, q] * Eb[l, q] via two replicate matmuls
            ap0 = e_sb[:].ap[0]
            ea_ap = bass.AP(e_sb.tensor, e_sb[:].offset + 1, [[ap0[0], L], [3, NT]])
            eb_ap = bass.AP(e_sb.tensor, e_sb[:].offset + 2, [[ap0[0], L], [3, NT]])
            rep_ps = ps_rep.tile([LL, 2, NT], f32)
            nc.tensor.matmul(rep_ps[:, 0], ra_sb[:], ea_ap, start=True, stop=True)
            nc.tensor.matmul(rep_ps[:, 1], rb_sb[:], eb_ap, start=True, stop=True)
            ea_cp = upool.tile([LL, NT], bf16)
            nc.scalar.copy(ea_cp[:], rep_ps[:, 0])
            u_sb = upool.tile([LL, NT], bf16)
            nc.vector.tensor_mul(u_sb[:], ea_cp[:], rep_ps[:, 1])

            # T^T [(i,j), q] = G6^T @ u  (triple-step transfer matrices)
            t_ps = ps_g6.tile([LL, NT], f32)
            nc.tensor.matmul(t_ps[:], g6_sb[:], u_sb[:], start=True, stop=True)
            nc.vector.tensor_copy(coll_sb[:, b, :], t_ps[:])

        nc.sync.dma_start(t_out[:], coll_sb[:])
        nc.sync.dma_start(emc_out[:], emc_sb[:])
        nc.scalar.dma_start(sm_out[:], small_sb[:])

    if not nc.is_finalized():
        nc.finalize()
    return nc


def _get_nc():
    if "nc" not in _CACHE:
        _CACHE["nc"] = _build_bass()
    return _CACHE["nc"]


def _host_consts(trans):
    expT = np.exp(trans.astype(np.float64))                      # [9,9] f64
    r = np.arange(LL)
    c = np.arange(LL)
    k = r // L
    l = r % L
    i = c // L
    j = c % L
    # G6[(k,l), (i,j)] = expT[i,k] * expT[k,l] * expT[l,j]
    g6 = (
        expT[i[None, :], k[:, None]]
        * expT[k[:, None], l[:, None]]
        * expT[l[:, None], j[None, :]]
    ).astype(ml_dtypes.bfloat16)
    ra = (k[None, :] == np.arange(L)[:, None]).astype(ml_dtypes.bfloat16)
    rb = (l[None, :] == np.arange(L)[:, None]).astype(ml_dtypes.bfloat16)
    iota = np.arange(L, dtype=np.float32).reshape(L, 1)
    return expT, g6, ra, rb, iota


def _numpy_reference(hs, mask, labels, W, bb, st, en, tr):
    # general fallback (only used when attention_mask is not all ones)
    em = hs.astype(np.float64) @ W.astype(np.float64) + bb.astype(np.float64)
    maskb = mask.astype(bool)
    maskf = mask.astype(np.float64)
    em_tag = np.take_along_axis(em, labels[..., None], axis=-1)[..., 0]
    num = st.astype(np.float64)[labels[:, 0]] + em_tag[:, 0]
    trs = tr.astype(np.float64)[labels[:, :-1], labels[:, 1:]]
    num = num + np.sum((trs + em_tag[:, 1:]) * maskf[:, 1:], axis=1)
    last = mask.sum(axis=1).astype(np.int64) - 1
    num = num + en.astype(np.float64)[labels[np.arange(len(labels)), last]]
    alpha = st.astype(np.float64)[None, :] + em[:, 0]
    for t in range(1, em.shape[1]):
        x = alpha[:, :, None] + tr.astype(np.float64)[None, :, :] + em[:, t][:, None, :]
        m = x.max(axis=1, keepdims=True)
        nxt = np.log(np.exp(x - m).sum(axis=1)) + m[:, 0, :]
        alpha = np.where(maskb[:, t][:, None], nxt, alpha)
    x = alpha + en.astype(np.float64)[None, :]
    m = x.max(axis=1, keepdims=True)
    denom = np.log(np.exp(x - m).sum(axis=1)) + m[:, 0]
    return np.asarray((denom - num).sum(), dtype=np.float32)


def _run_device(nc, in_maps):
    if os.environ.get("KERNEL_SIM"):
        from concourse.bass_interp import MultiCoreSim

        sim = MultiCoreSim(nc, len(in_maps))
        for t, m in enumerate(in_maps):
            for k2, v in m.items():
                sim.cores[t].tensor(k2)[:] = v
        sim.simulate()
        outs = []
        for t in range(len(in_maps)):
            outs.append(
                {
                    name: np.array(sim.cores[t].tensor(name))
                    for name in ("t_out", "emc_out", "sm_out")
                }
            )

        class _R:
            results = outs
            exec_time_ns = None

        return _R()
    from concourse import bass_utils

    return bass_utils.run_bass_kernel_spmd(nc, in_maps, list(range(len(in_maps))))


def kernel(**inputs):
    hs = np.asarray(inputs["hidden_states"], dtype=np.float32)
    mask = np.asarray(inputs["attention_mask"])
    labels = np.asarray(inputs["labels"]).astype(np.int64)
    W = np.asarray(inputs["W"], dtype=np.float32)
    bb = np.asarray(inputs["b"], dtype=np.float32)
    st = np.asarray(inputs["start_trans"], dtype=np.float32)
    en = np.asarray(inputs["end_trans"], dtype=np.float32)
    tr = np.asarray(inputs["trans"], dtype=np.float32)

    if not np.all(mask == 1):
        return _numpy_reference(hs, mask, labels, W, bb, st, en, tr)

    expT64, g6, ra, rb, iota = _host_consts(tr)

    # X -> fp8 e4m3 in [B, 128, HC, S] layout (h = 128*c + p)
    xq = np.clip(hs, -224.0, 224.0).astype(ml_dtypes.float8_e4m3)   # [B, S, H]
    x8 = np.ascontiguousarray(
        xq.transpose(0, 2, 1).reshape(B, HC, 128, S).transpose(0, 2, 1, 3)
    )                                                               # [B, 128, HC, S]
    wpad = np.zeros((H, MP), dtype=np.float32)
    wpad[:, :L] = W * SCALE_W
    w8 = np.ascontiguousarray(
        np.clip(wpad, -224.0, 224.0)
        .astype(ml_dtypes.float8_e4m3)
        .reshape(HC, 128, MP)
        .transpose(1, 0, 2)
    )                                                               # [128, HC, MP]
    labf = labels.astype(np.float32)
    lab9 = np.ascontiguousarray(np.broadcast_to(labf[:, None, :], (B, L, S)))
    bias9 = bb.reshape(L, 1).astype(np.float32)

    nc = _get_nc()
    in_maps = []
    for k in range(NCORES):
        sl = slice(k * BPC, (k + 1) * BPC)
        in_maps.append(
            {
                "x8": x8[sl],
                "w8": w8,
                "lab9": lab9[sl],
                "G6": g6,
                "Ra": ra,
                "Rb": rb,
                "iota9": iota,
                "bias9": bias9,
            }
        )
    res = _run_device(nc, in_maps)
    _CACHE["last_results"] = res

    # ---- host combine (f64, O(B * NT * 81)) ----
    st64 = st.astype(np.float64)
    en64 = en.astype(np.float64)
    e_en = np.exp(en64)
    e_st = np.exp(st64)
    total = 0.0
    for k in range(NCORES):
        r = res.results[k]
        Tm = (
            r["t_out"].astype(np.float64)
            .reshape(L, L, BPC, NT)
            .transpose(2, 3, 0, 1)
        )                                                   # [b, q, i, j]
        D = r["emc_out"].astype(np.float64).transpose(1, 2, 0)   # [b, q, j]
        M = Tm * D[:, :, None, :]
        logacc = np.zeros(BPC)
        while M.shape[1] > 1:
            n = M.shape[1]
            half = n // 2
            P = M[:, 0 : 2 * half : 2] @ M[:, 1 : 2 * half : 2]
            if n % 2:
                P = np.concatenate([P, M[:, 2 * half :]], axis=1)
            m = P.max(axis=(2, 3), keepdims=True)
            P /= m
            logacc += np.log(m[:, :, 0, 0]).sum(axis=1)
            M = P
        sm = r["sm_out"].astype(np.float64)                 # [9, b, 3]
        v0 = e_st[:, None] * sm[:, :, 0]                    # [9, b]
        v = np.einsum("jb,bjk->bk", v0, M[:, 0])
        v = (v @ expT64) * sm[:, :, 1].T                    # tail step t = S-1
        denom = np.log(v @ e_en) + logacc
        total += float(denom.sum())
        total -= float(sm[:, :, 2].sum()) / SCALE_W         # em_tag numerator
        lb = labels[k * BPC : (k + 1) * BPC]
        total -= float(
            st64[lb[:, 0]].sum()
            + en64[lb[:, -1]].sum()
            + tr.astype(np.float64)[lb[:, :-1], lb[:, 1:]].sum()
            + bb.astype(np.float64)[lb].sum()
        )
    return np.asarray(total, dtype=np.float32)


# revision 6
# speedup vs baseline: 3.4321x; 1.2862x over previous
"""BERT+CRF loss (torchcrf-style, reduction=sum) on 8 Trainium2 NeuronCores.

Strategy (pure data parallel, batch sharded 8 ways, 8 sequences per core):
  X is quantized to fp8-e4m3 on the host (4x less HBM traffic than f32) and
  streamed once through TensorE with DoubleRow fp8 matmuls (256-deep k-tiles)
  to produce emissions^T [9, 512] per sequence.  The CRF forward recurrence
  is reformulated in exp space: step matrix M_t[i,j] = expT[i,j] * E_t[j]
  with E_t = exp(em_t + b).  Triples of steps (t = 3q+2, 3q+3, 3q+4) collapse
  into one 9x9 transfer matrix via a bilinear host constant G6 [81, 81]:
      T_q[i,j] = sum_{k,l} expT[i,k] Ea[k] expT[k,l] Eb[l] expT[l,j]
  so  M_{3q+2} M_{3q+3} M_{3q+4} = T_q * diag(E_{3q+4}).
  The replicated log-space outer sum (ema[k]+emb[l]) is built with two
  accumulating indicator matmuls and exponentiated in one activation; a
  second matmul against G6 yields T^T [81, 170] per sequence.  The device
  ships the 170 T_q matrices (bf16) plus strided exp(em) columns; the host
  multiplies the 9x9 chain in f64 with an order-preserving normalized tree
  reduce (O(B*170*81) work) and adds the label-indexed numerator terms
  (accumulated on-device via a masked-sum op on the GpSimd engine).
  The per-sequence stages are software-pipelined with a skew of 2 so
  TensorE always has independent DoubleRow work queued.
"""

import os
import sys

if "/opt/trn_rl_repo" not in sys.path:
    sys.path.insert(0, "/opt/trn_rl_repo")

import ml_dtypes
import numpy as np

B, S, H, L = 64, 512, 768, 9
NCORES = 8
BPC = B // NCORES          # sequences per core
LL = L * L                 # 81
NT = 170                   # triples per sequence: steps t=2..511; t=1 on host
NE = 171                   # exp(em) column pairs shipped: cols 3q'+{0,1}
HC = H // 128              # 6 h-chunks of 128
NKT = 3                    # DoubleRow k-tiles (256-deep each)
MP = 16                    # DoubleRow needs >=16 weight cols per plane; 9 padded
SCALE_W = 64.0             # W is scaled into fp8 range; exp() unscales

_CACHE = {}


def _build_bass():
    import concourse.bass as bass
    import concourse.bacc as bacc
    import concourse.mybir as mybir
    import concourse.tile as tile
    from contextlib import ExitStack

    f32 = mybir.dt.float32
    bf16 = mybir.dt.bfloat16
    f8 = mybir.dt.float8e4
    Alu = mybir.AluOpType
    Act = mybir.ActivationFunctionType
    DR = mybir.MatmulPerfMode.DoubleRow

    nc = bacc.Bacc()

    # ---- I/O ----
    x8_d = nc.dram_tensor("x8", [BPC, 128, HC, S], f8, kind="ExternalInput")
    w8_d = nc.dram_tensor("w8", [128, HC, MP], f8, kind="ExternalInput")
    lab_d = nc.dram_tensor("lab9", [L, BPC, S], bf16, kind="ExternalInput")
    g6_d = nc.dram_tensor("G6", [LL, LL], bf16, kind="ExternalInput")
    rab_d = nc.dram_tensor("Rab", [L, 2, LL], bf16, kind="ExternalInput")
    iota_d = nc.dram_tensor("iota9", [L, 1], f32, kind="ExternalInput")
    bias_d = nc.dram_tensor("bias9", [L, 1], f32, kind="ExternalInput")
    bias81_d = nc.dram_tensor("bias81", [LL, 1], f32, kind="ExternalInput")

    t_out = nc.dram_tensor("t_out", [LL, BPC, NT], bf16, kind="ExternalOutput")
    eme_out = nc.dram_tensor("eme_out", [L, BPC, NE, 2], bf16, kind="ExternalOutput")
    nt_out = nc.dram_tensor("nt_out", [L, BPC], f32, kind="ExternalOutput")

    with ExitStack() as ctx:
        tc = ctx.enter_context(tile.TileContext(nc))
        const = ctx.enter_context(tc.tile_pool(name="const", bufs=1))
        xpool = ctx.enter_context(tc.tile_pool(name="x", bufs=3))
        epool = ctx.enter_context(tc.tile_pool(name="e", bufs=2))
        spool = ctx.enter_context(tc.tile_pool(name="sm", bufs=2))
        upool = ctx.enter_context(tc.tile_pool(name="u", bufs=2))
        rpool = ctx.enter_context(tc.tile_pool(name="res", bufs=1))
        ps_em = ctx.enter_context(tc.tile_pool(name="psem", bufs=3, space="PSUM"))
        ps_rep = ctx.enter_context(tc.tile_pool(name="psrep", bufs=2, space="PSUM"))
        ps_g6 = ctx.enter_context(tc.tile_pool(name="psg6", bufs=2, space="PSUM"))

        # ---- constants into SBUF ----
        w8_sb = const.tile([128, HC, MP], f8)
        nc.sync.dma_start(w8_sb[:], w8_d[:])
        g6_sb = const.tile([LL, LL], bf16)
        nc.sync.dma_start(g6_sb[:], g6_d[:])
        rab_sb = const.tile([L, 2, LL], bf16)
        nc.scalar.dma_start(rab_sb[:], rab_d[:])
        iota_sb = const.tile([L, 1], f32)
        nc.scalar.dma_start(iota_sb[:], iota_d[:])
        bias_sb = const.tile([L, 1], f32)
        nc.scalar.dma_start(bias_sb[:], bias_d[:])
        bias81_sb = const.tile([LL, 1], f32)
        nc.scalar.dma_start(bias81_sb[:], bias81_d[:])
        lab_sb = const.tile([L, BPC, S], bf16)
        nc.scalar.dma_start(lab_sb[:], lab_d[:])

        # ---- persistent result collect tiles ----
        coll_sb = rpool.tile([LL, BPC, NT], bf16)
        eme_sb = rpool.tile([L, BPC, NE, 2], bf16)
        ntag_sb = rpool.tile([L, BPC], f32)

        xts = [None] * BPC
        emps = [None] * BPC
        emsb = [None] * BPC

        def stage_a(b):
            # stream this sequence's X^T (fp8, per-partition contiguous 3KB)
            xt = xpool.tile([128, HC, S], f8)
            nc.sync.dma_start(xt[:], x8_d[b])
            xts[b] = xt
            # emissions^T [16, S] = (64*W)^T @ X^T via 3 DoubleRow k-tiles
            em_ps = ps_em.tile([MP, S], f32)
            for t in range(NKT):
                nc.tensor.matmul(
                    em_ps[:],
                    w8_sb[:, 2 * t : 2 * t + 2, :],
                    xt[:, 2 * t : 2 * t + 2, :],
                    start=(t == 0), stop=(t == NKT - 1),
                    perf_mode=DR,
                )
            emps[b] = em_ps

        def stage_b(b):
            em_ps = emps[b]
            # scaled emissions to SBUF (bf16) for the replicate matmuls
            em_sb = epool.tile([L, S], bf16)
            nc.vector.tensor_copy(em_sb[:], em_ps[0:L, :])
            emsb[b] = em_sb
            # exp(em) at columns 3q'+{0,1}: covers v0 (col 0), the host M_1
            # step (col 1) and the triple diag factors (cols 4,7,...,511)
            eme_ap = bass.AP(
                em_ps.tensor, em_ps[:].offset,
                [[em_ps[:].ap[0][0], L], [3, NE], [1, 2]],
            )
            nc.scalar.activation(
                eme_sb[:, b], eme_ap, Act.Exp, bias=bias_sb[:], scale=1.0 / SCALE_W
            )
            # numerator: sum_t em_scaled[label_t, t] accumulated per l
            msk = spool.tile([L, S], bf16)
            nc.vector.scalar_tensor_tensor(
                out=msk[:], in0=lab_sb[:, b], scalar=iota_sb[:], in1=em_sb[:],
                op0=Alu.is_equal, op1=Alu.mult,
                accum_out=ntag_sb[:, b : b + 1],
            )

        def stage_c(b):
            em_sb = emsb[b]
            # rep[(k,l), q] = ema[k, 3q+2] + emb[l, 3q+3] via two accumulating
            # indicator matmuls, then exp -> u (the G6 bilinear input)
            ap0 = em_sb[:].ap[0]
            ea_ap = bass.AP(em_sb.tensor, em_sb[:].offset + 2, [[ap0[0], L], [3, NT]])
            eb_ap = bass.AP(em_sb.tensor, em_sb[:].offset + 3, [[ap0[0], L], [3, NT]])
            rep_ps = ps_rep.tile([LL, NT], f32)
            nc.tensor.matmul(rep_ps[:], rab_sb[:, 0], ea_ap, start=True, stop=False)
            nc.tensor.matmul(rep_ps[:], rab_sb[:, 1], eb_ap, start=False, stop=True)
            u_sb = upool.tile([LL, NT], bf16)
            nc.scalar.activation(
                u_sb[:], rep_ps[:], Act.Exp, bias=bias81_sb[:], scale=1.0 / SCALE_W
            )
            # T^T [(i,j), q] = G6^T @ u  (triple-step transfer matrices)
            t_ps = ps_g6.tile([LL, NT], f32)
            nc.tensor.matmul(t_ps[:], g6_sb[:], u_sb[:], start=True, stop=True)
            nc.vector.tensor_copy(coll_sb[:, b, :], t_ps[:])
            xts[b] = emps[b] = emsb[b] = None

        stage_a(0)
        stage_a(1)
        for b in range(BPC):
            stage_b(b)
            stage_c(b)
            if b + 2 < BPC:
                stage_a(b + 2)

        nc.sync.dma_start(t_out[:], coll_sb[:])
        nc.sync.dma_start(eme_out[:], eme_sb[:])
        nc.scalar.dma_start(nt_out[:], ntag_sb[:])

    if not nc.is_finalized():
        nc.finalize()
    return nc


def _get_nc():
    if "nc" not in _CACHE:
        _CACHE["nc"] = _build_bass()
    return _CACHE["nc"]


def _host_consts(trans, bb):
    expT = np.exp(trans.astype(np.float64))                      # [9,9] f64
    r = np.arange(LL)
    c = np.arange(LL)
    k = r // L
    l = r % L
    i = c // L
    j = c % L
    # G6[(k,l), (i,j)] = expT[i,k] * expT[k,l] * expT[l,j]
    g6 = (
        expT[i[None, :], k[:, None]]
        * expT[k[:, None], l[:, None]]
        * expT[l[:, None], j[None, :]]
    ).astype(ml_dtypes.bfloat16)
    rab = np.zeros((L, 2, LL), dtype=ml_dtypes.bfloat16)
    rab[:, 0, :] = k[None, :] == np.arange(L)[:, None]
    rab[:, 1, :] = l[None, :] == np.arange(L)[:, None]
    iota = np.arange(L, dtype=np.float32).reshape(L, 1)
    b64 = bb.astype(np.float64)
    bias81 = (b64[k] + b64[l]).astype(np.float32).reshape(LL, 1)
    return expT, g6, rab, iota, bias81


def _numpy_reference(hs, mask, labels, W, bb, st, en, tr):
    # general fallback (only used when attention_mask is not all ones)
    em = hs.astype(np.float64) @ W.astype(np.float64) + bb.astype(np.float64)
    maskb = mask.astype(bool)
    maskf = mask.astype(np.float64)
    em_tag = np.take_along_axis(em, labels[..., None], axis=-1)[..., 0]
    num = st.astype(np.float64)[labels[:, 0]] + em_tag[:, 0]
    trs = tr.astype(np.float64)[labels[:, :-1], labels[:, 1:]]
    num = num + np.sum((trs + em_tag[:, 1:]) * maskf[:, 1:], axis=1)
    last = mask.sum(axis=1).astype(np.int64) - 1
    num = num + en.astype(np.float64)[labels[np.arange(len(labels)), last]]
    alpha = st.astype(np.float64)[None, :] + em[:, 0]
    for t in range(1, em.shape[1]):
        x = alpha[:, :, None] + tr.astype(np.float64)[None, :, :] + em[:, t][:, None, :]
        m = x.max(axis=1, keepdims=True)
        nxt = np.log(np.exp(x - m).sum(axis=1)) + m[:, 0, :]
        alpha = np.where(maskb[:, t][:, None], nxt, alpha)
    x = alpha + en.astype(np.float64)[None, :]
    m = x.max(axis=1, keepdims=True)
    denom = np.log(np.exp(x - m).sum(axis=1)) + m[:, 0]
    return np.asarray((denom - num).sum(), dtype=np.float32)


def _run_device(nc, in_maps):
    if os.environ.get("KERNEL_SIM"):
        from concourse.bass_interp import MultiCoreSim

        sim = MultiCoreSim(nc, len(in_maps))
        for t, m in enumerate(in_maps):
            for k2, v in m.items():
                sim.cores[t].tensor(k2)[:] = v
        sim.simulate()
        outs = []
        for t in range(len(in_maps)):
            outs.append(
                {
                    name: np.array(sim.cores[t].tensor(name))
                    for name in ("t_out", "eme_out", "nt_out")
                }
            )

        class _R:
            results = outs
            exec_time_ns = None

        return _R()
    from concourse import bass_utils

    return bass_utils.run_bass_kernel_spmd(nc, in_maps, list(range(len(in_maps))))


def kernel(**inputs):
    hs = np.asarray(inputs["hidden_states"], dtype=np.float32)
    mask = np.asarray(inputs["attention_mask"])
    labels = np.asarray(inputs["labels"]).astype(np.int64)
    W = np.asarray(inputs["W"], dtype=np.float32)
    bb = np.asarray(inputs["b"], dtype=np.float32)
    st = np.asarray(inputs["start_trans"], dtype=np.float32)
    en = np.asarray(inputs["end_trans"], dtype=np.float32)
    tr = np.asarray(inputs["trans"], dtype=np.float32)

    if not np.all(mask == 1):
        return _numpy_reference(hs, mask, labels, W, bb, st, en, tr)

    expT64, g6, rab, iota, bias81 = _host_consts(tr, bb)

    # X -> fp8 e4m3 in [B, 128, HC, S] layout (h = 128*c + p)
    xq = np.clip(hs, -224.0, 224.0).astype(ml_dtypes.float8_e4m3)   # [B, S, H]
    x8 = np.ascontiguousarray(
        xq.transpose(0, 2, 1).reshape(B, HC, 128, S).transpose(0, 2, 1, 3)
    )                                                               # [B, 128, HC, S]
    wpad = np.zeros((H, MP), dtype=np.float32)
    wpad[:, :L] = W * SCALE_W
    w8 = np.ascontiguousarray(
        np.clip(wpad, -224.0, 224.0)
        .astype(ml_dtypes.float8_e4m3)
        .reshape(HC, 128, MP)
        .transpose(1, 0, 2)
    )                                                               # [128, HC, MP]
    lab9 = np.ascontiguousarray(
        np.broadcast_to(
            labels.astype(ml_dtypes.bfloat16).reshape(NCORES, 1, BPC, S),
            (NCORES, L, BPC, S),
        )
    )                                                               # [NC, 9, BPC, S]
    bias9 = bb.reshape(L, 1).astype(np.float32)

    nc = _get_nc()
    in_maps = []
    for k in range(NCORES):
        sl = slice(k * BPC, (k + 1) * BPC)
        in_maps.append(
            {
                "x8": x8[sl],
                "w8": w8,
                "lab9": lab9[k],
                "G6": g6,
                "Rab": rab,
                "iota9": iota,
                "bias9": bias9,
                "bias81": bias81,
            }
        )
    res = _run_device(nc, in_maps)
    _CACHE["last_results"] = res

    # ---- host combine (f64, O(B * NT * 81)) ----
    st64 = st.astype(np.float64)
    en64 = en.astype(np.float64)
    e_en = np.exp(en64)
    e_st = np.exp(st64)
    total = 0.0
    for k in range(NCORES):
        r = res.results[k]
        Tm = (
            r["t_out"].astype(np.float64)
            .reshape(L, L, BPC, NT)
            .transpose(2, 3, 0, 1)
        )                                                   # [b, q, i, j]
        emE = r["eme_out"].astype(np.float64)               # [9, b, NE, 2]
        D = emE[:, :, 1:, 1].transpose(1, 2, 0)             # [b, q, j] diag factors
        M = Tm * D[:, :, None, :]
        logacc = np.zeros(BPC)
        while M.shape[1] > 1:
            n = M.shape[1]
            half = n // 2
            P = M[:, 0 : 2 * half : 2] @ M[:, 1 : 2 * half : 2]
            if n % 2:
                P = np.concatenate([P, M[:, 2 * half :]], axis=1)
            m = P.max(axis=(2, 3), keepdims=True)
            P /= m
            logacc += np.log(m[:, :, 0, 0]).sum(axis=1)
            M = P
        v0 = e_st[:, None] * emE[:, :, 0, 0]                # [9, b]
        v1 = (v0.T @ expT64) * emE[:, :, 0, 1].T            # host M_1 step [b, 9]
        v = np.einsum("bj,bjk->bk", v1, M[:, 0])
        denom = np.log(v @ e_en) + logacc
        total += float(denom.sum())
        total -= float(r["nt_out"].astype(np.float64).sum()) / SCALE_W
        lb = labels[k * BPC : (k + 1) * BPC]
        total -= float(
            st64[lb[:, 0]].sum()
            + en64[lb[:, -1]].sum()
            + tr.astype(np.float64)[lb[:, :-1], lb[:, 1:]].sum()
            + bb.astype(np.float64)[lb].sum()
        )
    return np.asarray(total, dtype=np.float32)
